# revision 13
# baseline (speedup 1.0000x reference)
"""Trainium2 kernel for nn_Circuit_41936060678727.

The reference is a 10-qubit real-amplitude circuit (CNOT ladders + RY
rotations), measured with PauliZ on each wire.  Every gate is linear, so the
circuit collapses to one 784x1024 matrix W with orthonormal rows:

    out[b, p] = sum_z sign_p(z) y_z^2 / ||y||^2,   y = W^T x_b

fp8 trick: x is uniform[0,1), so ~87% of each sample's energy sits in the
all-ones direction u.  Split x = m*u + xt (xt = x - mean, m = u^T x exact on
host).  Then with q = W^T u (||q||=1, exact):

    num_p = m^2 * alpha_p + 2 m * (gamma_p . xt) + sum_z s_pz yt_z^2
    den   = m^2 + ||yt||^2                     (u _|_ xt  =>  q _|_ yt exactly)

where alpha_p = sum_z s_pz q_z^2 (host, exact), gamma_p = W (s_p * q) (host,
quantized hi/lo fp8), yt = W^T xt.  Removing the mean shrinks the operand
~2.8x, which makes e4m3 DoubleRow matmuls (2x PE rate, K=256/pass) accurate
enough: simulated rel err 9.4e-3 vs the 2e-2 gate.

Device per core (batch 2048, groups of 512):
    mm1: yt = Wq^T x_hi          fp8 DR, K=1024(pad), 8 z-chunks, 128 MMs
    sq  = yt^2                   ACT/DVE split, fp8 out (|yt|<=11.6 -> sq<=135)
    mm2: po = [S|1]^T sq         fp8 DR, po strips share one PSUM bank whose
                                 has_written bits are pre-set by a zero matmul
    t2: gamma^T xt hi/lo trio    fp8 DR, M=32/16, transient PSUM tiles
Host: assemble num/den, divide.
"""

import numpy as np
import ml_dtypes

N_QUBITS = 10
DIM = 1 << N_QUBITS          # 1024
N_OUT = 10
D_IN = 784
B_TOTAL = 16384
N_CORES = 8
B_CORE = B_TOTAL // N_CORES  # 2048
GROUP = 512
N_GROUPS = B_CORE // GROUP   # 4
KSUB = 8                     # 1024 contraction rows = 8 subtiles of 128
NZ = 8                       # 1024 output states = 8 chunks of 128

S_X = 2.0                    # x_hi = e4m3(S_X * xt)
S_W = 4.0                    # Wq   = e4m3(S_W * W)   (S_X*S_W=8 keeps sq<240)
S_G = 16.0                   # g_hi = e4m3(S_G * gamma)
LO = 16.0                    # lo-residual upscale

F8 = ml_dtypes.float8_e4m3   # TRN FP8_EXP4 semantics (max 240, inf at 256)


# ----------------------------------------------------------------------------
# Host-side precompute
# ----------------------------------------------------------------------------

def _apply_ry(S, theta, q):
    B = S.shape[0]
    left, right = 1 << q, 1 << (N_QUBITS - q - 1)
    s = S.reshape(B, left, 2, right)
    c, sn = np.cos(theta / 2), np.sin(theta / 2)
    s0 = c * s[:, :, 0] - sn * s[:, :, 1]
    s1 = sn * s[:, :, 0] + c * s[:, :, 1]
    return np.stack([s0, s1], axis=2).reshape(B, DIM)


def _apply_cnot(S, q):
    B = S.shape[0]
    left, right = 1 << q, 1 << (N_QUBITS - q - 2)
    s = S.reshape(B, left, 2, 2, right)
    s = np.concatenate([s[:, :, :1], np.flip(s[:, :, 1:], axis=3)], axis=2)
    return s.reshape(B, DIM)


def _build_W(params):
    """Circuit applied to basis rows e_0..e_783 -> W[784, 1024], fp64."""
    w = np.pi * np.tanh(params.astype(np.float64))
    S = np.zeros((D_IN, DIM), dtype=np.float64)
    S[np.arange(D_IN), np.arange(D_IN)] = 1.0
    for l in range(params.shape[0]):
        for start in (0, 1):
            for i in range(start, N_QUBITS - 1, 2):
                S = _apply_cnot(S, i)
        for i in range(N_QUBITS):
            S = _apply_ry(S, w[l, i], i)
    return S


def _sign_matrix():
    z = np.arange(DIM)
    S = np.zeros((DIM, N_OUT), dtype=np.float64)
    for p in range(N_OUT):
        S[:, p] = 1.0 - 2.0 * ((z >> (N_QUBITS - 1 - p)) & 1)
    return S


def _e4(a):
    return np.asarray(a, np.float32).astype(F8)


def _pack_k_major(a):
    """[1024, cols] -> [128, 8, cols] with [p, s, :] = a[128 s + p, :]."""
    cols = a.shape[1]
    return np.ascontiguousarray(
        a.reshape(KSUB, 128, cols).transpose(1, 0, 2)
    )


# ----------------------------------------------------------------------------
# Bass program (identical SPMD program on all 8 cores)
# ----------------------------------------------------------------------------

_NC_CACHE = {}
TRACE = False
LAST_RESULTS = None


def _build_bass():
    from contextlib import ExitStack

    import concourse.tile as tile
    from concourse import bacc, mybir

    f32 = mybir.dt.float32
    f8 = mybir.dt.float8e4
    bf16 = mybir.dt.bfloat16
    DR = mybir.MatmulPerfMode.DoubleRow

    nc = bacc.Bacc(
        "TRN2", target_bir_lowering=False, debug=False, num_devices=N_CORES
    )
    zsb_d = nc.declare_dram_parameter("zsb", [128, KSUB, 16], bf16, isOutput=False)
    ghl_d = nc.declare_dram_parameter("ghl", [128, KSUB, 32], f8, isOutput=False)
    wrem_d = nc.declare_dram_parameter("wrem", [128, 2, 128], f8, isOutput=False)
    wq_d = nc.declare_dram_parameter("wq", [128, NZ * 6, 128], f8, isOutput=False)
    xh_ds = [
        nc.declare_dram_parameter(f"xh{q}", [128, 7, 2 * GROUP], f8, isOutput=False)
        for q in range(2)
    ]
    xl_ds = [
        nc.declare_dram_parameter(f"xl{q}", [128, 7, 2 * GROUP], f8, isOutput=False)
        for q in range(2)
    ]
    out_t2_d = nc.declare_dram_parameter("out_t2", [80, B_CORE], f32, isOutput=True)

    def mm(out, lhsT, rhs, start, stop, tile_position=None):
        nc.tensor.matmul(
            out, lhsT=lhsT, rhs=rhs, start=start, stop=stop,
            perf_mode=DR, skip_group_check=True, tile_position=tile_position,
        )

    with ExitStack() as ctx:
        tc = ctx.enter_context(tile.TileContext(nc))
        gz = ctx.enter_context(tc.tile_pool(name="gz", bufs=1))
        xpool = ctx.enter_context(tc.tile_pool(name="x", bufs=1))
        xlpool = ctx.enter_context(tc.tile_pool(name="xl", bufs=1))
        wpool = ctx.enter_context(tc.tile_pool(name="w", bufs=1))
        sqpool = ctx.enter_context(tc.tile_pool(name="sq", bufs=6))
        ybfpool = ctx.enter_context(tc.tile_pool(name="ybf", bufs=3))
        osb = ctx.enter_context(tc.tile_pool(name="osb", bufs=1))
        pypool = ctx.enter_context(tc.tile_pool(name="py", bufs=8, space="PSUM"))

        # ring B (scalar HWDGE): weights + small tensors
        ghl = gz.tile([128, KSUB, 32], f8, tag="ghl")
        nc.scalar.dma_start(ghl[:], ghl_d[:, :, :])
        zsb = gz.tile([128, KSUB, 16], bf16, tag="zsb")
        nc.scalar.dma_start(zsb[:], zsb_d[:, :, :])
        wrem = gz.tile([128, 2, 128], f8, tag="wrem")
        nc.scalar.dma_start(wrem[:], wrem_d[:, :, :])
        wq_sb = wpool.tile([128, 48, 128], f8, tag="wq")
        nc.scalar.dma_start(wq_sb[:], wq_d[:, :, :])
        wz = gz.tile([128, 128], f8, tag="wz")
        nc.vector.memset(wz[:], 0.0)

        # ring A (sync HWDGE): x hi/lo in group-pairs (7 KB partition lines)
        xp_sb, xlp_sb = [], []
        for q in range(2):
            t = xpool.tile([128, 7, 2 * GROUP], f8, tag=f"xh{q}", name=f"xh{q}")
            nc.sync.dma_start(t[:], xh_ds[q][:, :, :])
            xp_sb.append(t)
            t = xlpool.tile([128, 7, 2 * GROUP], f8, tag=f"xl{q}", name=f"xl{q}")
            nc.sync.dma_start(t[:], xl_ds[q][:, :, :])
            xlp_sb.append(t)

        def xs(g, p0, s, p1=None):
            t = xp_sb[g // 2]
            c0 = (g % 2) * GROUP
            if p1 is None:
                return t[:, s, c0:c0 + GROUP] if isinstance(s, int) else None
            return t[p0:p1, s, c0:c0 + GROUP]

        def wslice(z, c):
            return wq_sb[:, z * 6 + 2 * c:z * 6 + 2 * c + 2, :]

        out_t2_sb = osb.tile([80, B_CORE], f32, tag="ot2")

        # --- PE warmup on framework const tiles (no DMA dependency) ---
        warm = pypool.tile([128, GROUP], f32, tag="py", name="warm")
        cbf = nc.const_aps.aps[(mybir.dt.bfloat16, 1.0)]
        for _ in range(9):
            nc.tensor.matmul(
                warm[0:1, :], lhsT=cbf, rhs=cbf.to_broadcast([128, GROUP]),
                start=True, stop=True, skip_group_check=True,
            )

        sq_tiles = {}

        def emit_mm1(g):
            # two z-quads: 12 DR passes (K=768) + 4 row-packed K=16 remainders
            for h in range(2):
                pys = [
                    pypool.tile([128, GROUP], f32, tag="py", name=f"py{g}_{4*h+r}")
                    for r in range(4)
                ]
                c0 = (g % 2) * GROUP
                xt_ = xp_sb[g // 2]
                for c in range(3):
                    for r in range(4):
                        mm(pys[r][:], wslice(4 * h + r, c),
                           xt_[:, 2 * c:2 * c + 2, c0:c0 + GROUP],
                           start=(c == 0), stop=False)
                for r in range(4):
                    nc.tensor.matmul(
                        pys[r][:], lhsT=wrem[32 * r:32 * r + 16, h, :],
                        rhs=xt_[32 * r:32 * r + 16, 6, c0:c0 + GROUP],
                        start=False, stop=True, skip_group_check=True,
                        tile_position=(32 * r, 0),
                    )
                for r in range(4):
                    z = 4 * h + r
                    pair = z // 2
                    if z % 2 == 0:
                        sq_tiles[(pair, g)] = sqpool.tile(
                            [128, 2, GROUP], bf16, tag="sqb", name=f"sq{pair}_{g}"
                        )
                    dst = sq_tiles[(pair, g)][:, z % 2, :]
                    if z % 2 == 0:
                        nc.scalar.square(dst, pys[r][:])
                    else:
                        ybf = ybfpool.tile([128, GROUP], bf16, tag="ybf",
                                           name=f"ybf{g}_{z}")
                        nc.vector.tensor_copy(ybf[:], pys[r][:])
                        nc.vector.tensor_tensor(dst, ybf[:], ybf[:],
                                                mybir.AluOpType.mult)

        def emit_trio(g):
            # one PSUM bank: zero-matmul sets has_written for the whole bank,
            # then t2a (col grp 0), t2b (col grp 1), mm2 (col grp 2) chains
            # accumulate with start=False; consecutive MMs hit disjoint column
            # groups and run concurrently.
            trio = pypool.tile([128, GROUP], f32, tag="py", name=f"trio{g}")
            c0 = (g % 2) * GROUP
            xt_ = xp_sb[g // 2]
            xlt = xlp_sb[g // 2]
            gcols = slice(g * GROUP, (g + 1) * GROUP)
            nc.tensor.matmul(
                trio[:], lhsT=wz[:], rhs=xt_[:, 0, c0:c0 + GROUP],
                start=True, stop=False, skip_group_check=True,
            )
            for s in range(8):
                if s < 6:
                    nc.tensor.matmul(
                        trio[0:32, :], lhsT=ghl[:, s, 0:32],
                        rhs=xt_[:, s, c0:c0 + GROUP],
                        start=False, stop=False, skip_group_check=True,
                        tile_position=(0, 0),
                    )
                    nc.tensor.matmul(
                        trio[32:48, :], lhsT=ghl[:, s, 0:16],
                        rhs=xlt[:, s, c0:c0 + GROUP],
                        start=False, stop=False, skip_group_check=True,
                        tile_position=(0, 32),
                    )
                elif s == 6:  # K=16 tails (rows 768..783)
                    nc.tensor.matmul(
                        trio[0:32, :], lhsT=ghl[0:16, 6, 0:32],
                        rhs=xt_[0:16, 6, c0:c0 + GROUP],
                        start=False, stop=True, skip_group_check=True,
                        tile_position=(0, 0),
                    )
                    nc.tensor.matmul(
                        trio[32:48, :], lhsT=ghl[0:16, 6, 0:16],
                        rhs=xlt[0:16, 6, c0:c0 + GROUP],
                        start=False, stop=True, skip_group_check=True,
                        tile_position=(0, 32),
                    )
                sqt = sq_tiles.pop((s // 2, g)) if s % 2 == 0 else sqt_hold
                sqt_hold = sqt
                nc.tensor.matmul(
                    trio[64:80, :], lhsT=zsb[:, s, :], rhs=sqt[:, s % 2, :],
                    start=False, stop=(s == 7), skip_group_check=True,
                    tile_position=(0, 64),
                )
            if g % 2 == 0:
                nc.scalar.copy(out_t2_sb[0:80, gcols], trio[0:80, :])
            else:
                nc.vector.tensor_copy(out_t2_sb[0:80, gcols], trio[0:80, :])
            nc.gpsimd.dma_start(out_t2_d[:, gcols], out_t2_sb[:, gcols])

        for g in range(N_GROUPS):
            emit_mm1(g)
            emit_trio(g)

    nc.finalize()
    return nc


def _get_nc():
    if "nc" not in _NC_CACHE:
        _NC_CACHE["nc"] = _build_bass()
    return _NC_CACHE["nc"]


# ----------------------------------------------------------------------------
# Entry point
# ----------------------------------------------------------------------------

def kernel(input, params):
    global LAST_RESULTS
    from concourse.bass_utils import run_bass_kernel_spmd

    x = np.asarray(input, dtype=np.float64)
    p = np.asarray(params, dtype=np.float32)
    B = x.shape[0]
    assert B == B_TOTAL and x.shape[1] == D_IN

    W = _build_W(p)                       # [784, 1024] fp64
    S = _sign_matrix()                    # [1024, 10]
    u = np.ones(D_IN) / np.sqrt(D_IN)
    q = W.T @ u                           # [1024]
    alpha = (S * (q**2)[:, None]).sum(axis=0)          # [10]
    gamma = W @ (S * q[:, None])                       # [784, 10]

    # per-sample DC split (host, exact fp64)
    m = x @ u                                           # [B]
    xt = (x - m[:, None] * u[None, :]).astype(np.float32)

    # quantize + pack weights
    Wp = np.zeros((DIM, DIM), dtype=np.float32)
    Wp[:D_IN] = S_W * W.astype(np.float32)
    W8 = _e4(Wp)                                        # [1024, 1024]
    # wq[p, z*6+s, m] = W8[128 s + p, 128 z + m] for s < 6 (K rows 0..767)
    wq_host = np.ascontiguousarray(
        W8.reshape(KSUB, 128, NZ, 128).transpose(1, 2, 0, 3)[:, :, 0:6, :]
        .reshape(128, NZ * 6, 128)
    )
    # wrem[32 r + j, h, m] = W8[768 + j, 128 (4 h + r) + m]  (K rows 768..783)
    wrem_host = np.zeros((128, 2, 128), dtype=W8.dtype)
    for r in range(4):
        for h in range(2):
            wrem_host[32 * r:32 * r + 16, h, :] = W8[768:784,
                                                     128 * (4 * h + r):
                                                     128 * (4 * h + r) + 128]

    G = np.zeros((DIM, 32), dtype=np.float32)
    G[:D_IN, 0:N_OUT] = S_G * gamma
    g_hi = _e4(G)
    g_res = np.zeros((DIM, 32), dtype=np.float32)
    g_res[:, 16:16 + N_OUT] = LO * (
        G[:, 0:N_OUT] - g_hi[:, 0:N_OUT].astype(np.float32)
    )
    ghl_host = _pack_k_major(
        (g_hi.astype(np.float32) + g_res).astype(np.float32)
    )
    ghl_host = _e4(ghl_host)

    Z = np.zeros((DIM, 16), dtype=np.float32)
    Z[:, :N_OUT] = S
    Z[:, N_OUT] = 1.0
    zsb_host = _pack_k_major(Z).astype(ml_dtypes.bfloat16)

    # quantize + pack x (hi/lo): [128, 7, B] with ksub 6 = K rows 768..783
    # at partition strips 0,32,64,96 (replicas feed the row-packed remainder)
    xtT = np.zeros((DIM, B), dtype=np.float32)
    xtT[:D_IN] = S_X * xt.T
    x_hi = _e4(xtT)                                     # [1024, B]
    x_lo = _e4(LO * (xtT - x_hi.astype(np.float32)))

    def pack_x(a):
        out = np.zeros((128, 7, B), dtype=a.dtype)
        out[:, 0:6, :] = a[0:768].reshape(6, 128, B).transpose(1, 0, 2)
        for r in range(4):
            out[32 * r:32 * r + 16, 6, :] = a[768:784]
        return out

    xh_all = pack_x(x_hi)
    xl_all = pack_x(x_lo)

    nc = _get_nc()
    in_maps = []
    for c in range(N_CORES):
        sl = slice(c * B_CORE, (c + 1) * B_CORE)
        m_in = {
            "zsb": zsb_host,
            "ghl": ghl_host,
            "wrem": wrem_host,
            "wq": wq_host,
        }
        for q in range(2):
            qsl = slice(c * B_CORE + q * 2 * GROUP,
                        c * B_CORE + (q + 1) * 2 * GROUP)
            m_in[f"xh{q}"] = np.ascontiguousarray(xh_all[:, :, qsl])
            m_in[f"xl{q}"] = np.ascontiguousarray(xl_all[:, :, qsl])
        in_maps.append(m_in)

    res = run_bass_kernel_spmd(nc, in_maps, list(range(N_CORES)), trace=TRACE)
    LAST_RESULTS = res

    SC2 = (S_X * S_W) ** 2
    outs = np.empty((B, N_OUT), dtype=np.float64)
    for c in range(N_CORES):
        t2r = res.results[c]["out_t2"].astype(np.float64)   # [64, 2048]
        t3 = t2r[64:64 + N_OUT].T / SC2
        n3 = t2r[64 + N_OUT] / SC2
        t2 = (
            t2r[0:N_OUT].T / (S_X * S_G)
            + (t2r[16:16 + N_OUT].T + t2r[32:32 + N_OUT].T) / (LO * S_X * S_G)
        )
        mc = m[c * B_CORE:(c + 1) * B_CORE]
        num = mc[:, None] ** 2 * alpha[None, :] + 2 * mc[:, None] * t2 + t3
        den = mc**2 + n3
        outs[c * B_CORE:(c + 1) * B_CORE] = num / den[:, None]

    return np.ascontiguousarray(outs.astype(np.float32))


# revision 14
# speedup vs baseline: 1.0900x; 1.0900x over previous
"""Trainium2 kernel for nn_Circuit_41936060678727.

The reference is a 10-qubit real-amplitude circuit (CNOT ladders + RY
rotations), measured with PauliZ on each wire.  Every gate is linear, so the
circuit collapses to one 784x1024 matrix W with orthonormal rows:

    out[b, p] = sum_z sign_p(z) y_z^2 / ||y||^2,   y = W^T x_b

fp8 trick: x is uniform[0,1), so ~87% of each sample's energy sits in the
all-ones direction u.  Split x = m*u + xt (xt = x - mean, m = u^T x exact on
host).  Then with q = W^T u (||q||=1, exact):

    num_p = m^2 * alpha_p + 2 m * (gamma_p . xt) + sum_z s_pz yt_z^2
    den   = m^2 + ||yt||^2                     (u _|_ xt  =>  q _|_ yt exactly)

where alpha_p = sum_z s_pz q_z^2 (host, exact), gamma_p = W (s_p * q) (host,
quantized hi/lo fp8), yt = W^T xt.  Removing the mean shrinks the operand
~2.8x, which makes e4m3 DoubleRow matmuls (2x PE rate, K=256/pass) accurate
enough: simulated rel err 9.4e-3 vs the 2e-2 gate.

Device per core (batch 2048, groups of 512):
    mm1: yt = Wq^T x_hi          fp8 DR, K=1024(pad), 8 z-chunks, 128 MMs
    sq  = yt^2                   ACT/DVE split, fp8 out (|yt|<=11.6 -> sq<=135)
    mm2: po = [S|1]^T sq         fp8 DR, po strips share one PSUM bank whose
                                 has_written bits are pre-set by a zero matmul
    t2: gamma^T xt hi/lo trio    fp8 DR, M=32/16, transient PSUM tiles
Host: assemble num/den, divide.
"""

import numpy as np
import ml_dtypes

N_QUBITS = 10
DIM = 1 << N_QUBITS          # 1024
N_OUT = 10
D_IN = 784
B_TOTAL = 16384
N_CORES = 8
B_CORE = B_TOTAL // N_CORES  # 2048
GROUP = 512
N_GROUPS = B_CORE // GROUP   # 4
KSUB = 8                     # 1024 contraction rows = 8 subtiles of 128
NZ = 8                       # 1024 output states = 8 chunks of 128

S_X = 2.0                    # x_hi = e4m3(S_X * xt)
S_W = 4.0                    # Wq   = e4m3(S_W * W)   (S_X*S_W=8 keeps sq<240)
S_G = 16.0                   # g_hi = e4m3(S_G * gamma)
LO = 16.0                    # lo-residual upscale

F8 = ml_dtypes.float8_e4m3   # TRN FP8_EXP4 semantics (max 240, inf at 256)


# ----------------------------------------------------------------------------
# Host-side precompute
# ----------------------------------------------------------------------------

def _apply_ry(S, theta, q):
    B = S.shape[0]
    left, right = 1 << q, 1 << (N_QUBITS - q - 1)
    s = S.reshape(B, left, 2, right)
    c, sn = np.cos(theta / 2), np.sin(theta / 2)
    s0 = c * s[:, :, 0] - sn * s[:, :, 1]
    s1 = sn * s[:, :, 0] + c * s[:, :, 1]
    return np.stack([s0, s1], axis=2).reshape(B, DIM)


def _apply_cnot(S, q):
    B = S.shape[0]
    left, right = 1 << q, 1 << (N_QUBITS - q - 2)
    s = S.reshape(B, left, 2, 2, right)
    s = np.concatenate([s[:, :, :1], np.flip(s[:, :, 1:], axis=3)], axis=2)
    return s.reshape(B, DIM)


def _build_W(params):
    """Circuit applied to basis rows e_0..e_783 -> W[784, 1024], fp64."""
    w = np.pi * np.tanh(params.astype(np.float64))
    S = np.zeros((D_IN, DIM), dtype=np.float64)
    S[np.arange(D_IN), np.arange(D_IN)] = 1.0
    for l in range(params.shape[0]):
        for start in (0, 1):
            for i in range(start, N_QUBITS - 1, 2):
                S = _apply_cnot(S, i)
        for i in range(N_QUBITS):
            S = _apply_ry(S, w[l, i], i)
    return S


def _sign_matrix():
    z = np.arange(DIM)
    S = np.zeros((DIM, N_OUT), dtype=np.float64)
    for p in range(N_OUT):
        S[:, p] = 1.0 - 2.0 * ((z >> (N_QUBITS - 1 - p)) & 1)
    return S


def _e4(a):
    return np.asarray(a, np.float32).astype(F8)


def _pack_k_major(a):
    """[1024, cols] -> [128, 8, cols] with [p, s, :] = a[128 s + p, :]."""
    cols = a.shape[1]
    return np.ascontiguousarray(
        a.reshape(KSUB, 128, cols).transpose(1, 0, 2)
    )


# ----------------------------------------------------------------------------
# Bass program (identical SPMD program on all 8 cores)
# ----------------------------------------------------------------------------

_NC_CACHE = {}
TRACE = False
LAST_RESULTS = None


def _build_bass():
    from contextlib import ExitStack

    import concourse.tile as tile
    from concourse import bacc, mybir

    f32 = mybir.dt.float32
    f8 = mybir.dt.float8e4
    bf16 = mybir.dt.bfloat16
    DR = mybir.MatmulPerfMode.DoubleRow

    nc = bacc.Bacc(
        "TRN2", target_bir_lowering=False, debug=False, num_devices=N_CORES
    )
    zsb_d = nc.declare_dram_parameter("zsb", [128, KSUB, 16], bf16, isOutput=False)
    ghl_d = nc.declare_dram_parameter("ghl", [128, KSUB, 32], f8, isOutput=False)
    wrem_d = nc.declare_dram_parameter("wrem", [128, 2, 128], f8, isOutput=False)
    wq_d = nc.declare_dram_parameter("wq", [128, NZ * 6, 128], f8, isOutput=False)
    xh_ds = [
        nc.declare_dram_parameter(f"xh{q}", [128, 7, 2 * GROUP], f8, isOutput=False)
        for q in range(2)
    ]
    xl_ds = [
        nc.declare_dram_parameter(f"xl{q}", [128, 7, 2 * GROUP], f8, isOutput=False)
        for q in range(2)
    ]
    out_t2_d = nc.declare_dram_parameter("out_t2", [80, B_CORE], f32, isOutput=True)

    def mm(out, lhsT, rhs, start, stop, tile_position=None):
        nc.tensor.matmul(
            out, lhsT=lhsT, rhs=rhs, start=start, stop=stop,
            perf_mode=DR, skip_group_check=True, tile_position=tile_position,
        )

    with ExitStack() as ctx:
        tc = ctx.enter_context(tile.TileContext(nc))
        gz = ctx.enter_context(tc.tile_pool(name="gz", bufs=1))
        xpool = ctx.enter_context(tc.tile_pool(name="x", bufs=1))
        xlpool = ctx.enter_context(tc.tile_pool(name="xl", bufs=1))
        wpool = ctx.enter_context(tc.tile_pool(name="w", bufs=1))
        sqpool = ctx.enter_context(tc.tile_pool(name="sq", bufs=6))
        ybfpool = ctx.enter_context(tc.tile_pool(name="ybf", bufs=3))
        osb = ctx.enter_context(tc.tile_pool(name="osb", bufs=1))
        pypool = ctx.enter_context(tc.tile_pool(name="py", bufs=8, space="PSUM"))

        # ring B (scalar HWDGE): weights + small tensors
        ghl = gz.tile([128, KSUB, 32], f8, tag="ghl")
        nc.scalar.dma_start(ghl[:], ghl_d[:, :, :])
        zsb = gz.tile([128, KSUB, 16], bf16, tag="zsb")
        nc.scalar.dma_start(zsb[:], zsb_d[:, :, :])
        wrem = gz.tile([128, 2, 128], f8, tag="wrem")
        nc.scalar.dma_start(wrem[:], wrem_d[:, :, :])
        wz = gz.tile([128, 128], f8, tag="wz")
        nc.vector.memset(wz[:], 0.0)

        # ring A (sync HWDGE) carries all big tensors in consumption order;
        # the scalar ring only has the small ones (packet round-robin would
        # starve a big transfer with small partition lines on ring B).
        xp_sb, xlp_sb = [], []
        t = xpool.tile([128, 7, 2 * GROUP], f8, tag="xh0", name="xh0")
        nc.sync.dma_start(t[:], xh_ds[0][:, :, :])
        xp_sb.append(t)
        wq_sb = wpool.tile([128, 48, 128], f8, tag="wq")
        nc.sync.dma_start(wq_sb[:], wq_d[:, :, :])
        t = xpool.tile([128, 7, 2 * GROUP], f8, tag="xh1", name="xh1")
        nc.sync.dma_start(t[:], xh_ds[1][:, :, :])
        xp_sb.append(t)
        for q in range(2):
            t = xlpool.tile([128, 7, 2 * GROUP], f8, tag=f"xl{q}", name=f"xl{q}")
            nc.sync.dma_start(t[:], xl_ds[q][:, :, :])
            xlp_sb.append(t)

        def xs(g, p0, s, p1=None):
            t = xp_sb[g // 2]
            c0 = (g % 2) * GROUP
            if p1 is None:
                return t[:, s, c0:c0 + GROUP] if isinstance(s, int) else None
            return t[p0:p1, s, c0:c0 + GROUP]

        def wslice(z, c):
            return wq_sb[:, z * 6 + 2 * c:z * 6 + 2 * c + 2, :]

        out_t2_sb = osb.tile([80, B_CORE], f32, tag="ot2")

        # --- PE warmup on framework const tiles (no DMA dependency) ---
        warm = pypool.tile([128, GROUP], f32, tag="py", name="warm")
        cbf = nc.const_aps.aps[(mybir.dt.bfloat16, 1.0)]
        for _ in range(9):
            nc.tensor.matmul(
                warm[0:1, :], lhsT=cbf, rhs=cbf.to_broadcast([128, GROUP]),
                start=True, stop=True, skip_group_check=True,
            )

        sq_tiles = {}

        def emit_mm1(g):
            # two z-quads: 12 DR passes (K=768) + 4 row-packed K=16 remainders
            for h in range(2):
                pys = [
                    pypool.tile([128, GROUP], f32, tag="py", name=f"py{g}_{4*h+r}")
                    for r in range(4)
                ]
                c0 = (g % 2) * GROUP
                xt_ = xp_sb[g // 2]
                for c in range(3):
                    for r in range(4):
                        mm(pys[r][:], wslice(4 * h + r, c),
                           xt_[:, 2 * c:2 * c + 2, c0:c0 + GROUP],
                           start=(c == 0), stop=False)
                for r in range(4):
                    nc.tensor.matmul(
                        pys[r][:], lhsT=wrem[32 * r:32 * r + 16, h, :],
                        rhs=xt_[32 * r:32 * r + 16, 6, c0:c0 + GROUP],
                        start=False, stop=True, skip_group_check=True,
                        tile_position=(32 * r, 0),
                    )
                for r in range(4):
                    z = 4 * h + r
                    pair = z // 2
                    if z % 2 == 0:
                        sq_tiles[(pair, g)] = sqpool.tile(
                            [128, 2, GROUP], bf16, tag="sqb", name=f"sq{pair}_{g}"
                        )
                    dst = sq_tiles[(pair, g)][:, z % 2, :]
                    if z % 2 == 0:
                        nc.scalar.square(dst, pys[r][:])
                    else:
                        ybf = ybfpool.tile([128, GROUP], bf16, tag="ybf",
                                           name=f"ybf{g}_{z}")
                        nc.vector.tensor_copy(ybf[:], pys[r][:])
                        nc.vector.tensor_tensor(dst, ybf[:], ybf[:],
                                                mybir.AluOpType.mult)

        def emit_trio(g):
            # one PSUM bank: zero-matmul sets has_written for the whole bank,
            # then t2a (col grp 0), t2b (col grp 1), mm2 (col grp 2) chains
            # accumulate with start=False; consecutive MMs hit disjoint column
            # groups and run concurrently.
            trio = pypool.tile([128, GROUP], f32, tag="py", name=f"trio{g}")
            c0 = (g % 2) * GROUP
            xt_ = xp_sb[g // 2]
            xlt = xlp_sb[g // 2]
            gcols = slice(g * GROUP, (g + 1) * GROUP)
            nc.tensor.matmul(
                trio[:], lhsT=wz[:], rhs=xt_[:, 0, c0:c0 + GROUP],
                start=True, stop=False, skip_group_check=True,
            )
            for s in range(8):
                if s < 6:
                    nc.tensor.matmul(
                        trio[0:32, :], lhsT=ghl[:, s, 0:32],
                        rhs=xt_[:, s, c0:c0 + GROUP],
                        start=False, stop=False, skip_group_check=True,
                        tile_position=(0, 0),
                    )
                    nc.tensor.matmul(
                        trio[32:48, :], lhsT=ghl[:, s, 0:16],
                        rhs=xlt[:, s, c0:c0 + GROUP],
                        start=False, stop=False, skip_group_check=True,
                        tile_position=(0, 32),
                    )
                elif s == 6:  # K=16 tails (rows 768..783)
                    nc.tensor.matmul(
                        trio[0:32, :], lhsT=ghl[0:16, 6, 0:32],
                        rhs=xt_[0:16, 6, c0:c0 + GROUP],
                        start=False, stop=True, skip_group_check=True,
                        tile_position=(0, 0),
                    )
                    nc.tensor.matmul(
                        trio[32:48, :], lhsT=ghl[0:16, 6, 0:16],
                        rhs=xlt[0:16, 6, c0:c0 + GROUP],
                        start=False, stop=True, skip_group_check=True,
                        tile_position=(0, 32),
                    )
                sqt = sq_tiles.pop((s // 2, g)) if s % 2 == 0 else sqt_hold
                sqt_hold = sqt
                nc.tensor.matmul(
                    trio[64:80, :], lhsT=zsb[:, s, :], rhs=sqt[:, s % 2, :],
                    start=False, stop=(s == 7), skip_group_check=True,
                    tile_position=(0, 64),
                )
            if g % 2 == 0:
                nc.scalar.copy(out_t2_sb[0:80, gcols], trio[0:80, :])
            else:
                nc.vector.tensor_copy(out_t2_sb[0:80, gcols], trio[0:80, :])
            nc.gpsimd.dma_start(out_t2_d[:, gcols], out_t2_sb[:, gcols])

        for g in range(N_GROUPS):
            emit_mm1(g)
            emit_trio(g)

    nc.finalize()
    return nc


def _get_nc():
    if "nc" not in _NC_CACHE:
        _NC_CACHE["nc"] = _build_bass()
    return _NC_CACHE["nc"]


# ----------------------------------------------------------------------------
# Entry point
# ----------------------------------------------------------------------------

def kernel(input, params):
    global LAST_RESULTS
    from concourse.bass_utils import run_bass_kernel_spmd

    x = np.asarray(input, dtype=np.float64)
    p = np.asarray(params, dtype=np.float32)
    B = x.shape[0]
    assert B == B_TOTAL and x.shape[1] == D_IN

    W = _build_W(p)                       # [784, 1024] fp64
    S = _sign_matrix()                    # [1024, 10]
    u = np.ones(D_IN) / np.sqrt(D_IN)
    q = W.T @ u                           # [1024]
    alpha = (S * (q**2)[:, None]).sum(axis=0)          # [10]
    gamma = W @ (S * q[:, None])                       # [784, 10]

    # per-sample DC split (host, exact fp64)
    m = x @ u                                           # [B]
    xt = (x - m[:, None] * u[None, :]).astype(np.float32)

    # quantize + pack weights
    Wp = np.zeros((DIM, DIM), dtype=np.float32)
    Wp[:D_IN] = S_W * W.astype(np.float32)
    W8 = _e4(Wp)                                        # [1024, 1024]
    # wq[p, z*6+s, m] = W8[128 s + p, 128 z + m] for s < 6 (K rows 0..767)
    wq_host = np.ascontiguousarray(
        W8.reshape(KSUB, 128, NZ, 128).transpose(1, 2, 0, 3)[:, :, 0:6, :]
        .reshape(128, NZ * 6, 128)
    )
    # wrem[32 r + j, h, m] = W8[768 + j, 128 (4 h + r) + m]  (K rows 768..783)
    wrem_host = np.zeros((128, 2, 128), dtype=W8.dtype)
    for r in range(4):
        for h in range(2):
            wrem_host[32 * r:32 * r + 16, h, :] = W8[768:784,
                                                     128 * (4 * h + r):
                                                     128 * (4 * h + r) + 128]

    G = np.zeros((DIM, 32), dtype=np.float32)
    G[:D_IN, 0:N_OUT] = S_G * gamma
    g_hi = _e4(G)
    g_res = np.zeros((DIM, 32), dtype=np.float32)
    g_res[:, 16:16 + N_OUT] = LO * (
        G[:, 0:N_OUT] - g_hi[:, 0:N_OUT].astype(np.float32)
    )
    ghl_host = _pack_k_major(
        (g_hi.astype(np.float32) + g_res).astype(np.float32)
    )
    ghl_host = _e4(ghl_host)

    Z = np.zeros((DIM, 16), dtype=np.float32)
    Z[:, :N_OUT] = S
    Z[:, N_OUT] = 1.0
    zsb_host = _pack_k_major(Z).astype(ml_dtypes.bfloat16)

    # quantize + pack x (hi/lo): [128, 7, B] with ksub 6 = K rows 768..783
    # at partition strips 0,32,64,96 (replicas feed the row-packed remainder)
    xtT = np.zeros((DIM, B), dtype=np.float32)
    xtT[:D_IN] = S_X * xt.T
    x_hi = _e4(xtT)                                     # [1024, B]
    x_lo = _e4(LO * (xtT - x_hi.astype(np.float32)))

    def pack_x(a):
        out = np.zeros((128, 7, B), dtype=a.dtype)
        out[:, 0:6, :] = a[0:768].reshape(6, 128, B).transpose(1, 0, 2)
        for r in range(4):
            out[32 * r:32 * r + 16, 6, :] = a[768:784]
        return out

    xh_all = pack_x(x_hi)
    xl_all = pack_x(x_lo)

    nc = _get_nc()
    in_maps = []
    for c in range(N_CORES):
        sl = slice(c * B_CORE, (c + 1) * B_CORE)
        m_in = {
            "zsb": zsb_host,
            "ghl": ghl_host,
            "wrem": wrem_host,
            "wq": wq_host,
        }
        for q in range(2):
            qsl = slice(c * B_CORE + q * 2 * GROUP,
                        c * B_CORE + (q + 1) * 2 * GROUP)
            m_in[f"xh{q}"] = np.ascontiguousarray(xh_all[:, :, qsl])
            m_in[f"xl{q}"] = np.ascontiguousarray(xl_all[:, :, qsl])
        in_maps.append(m_in)

    res = run_bass_kernel_spmd(nc, in_maps, list(range(N_CORES)), trace=TRACE)
    LAST_RESULTS = res

    SC2 = (S_X * S_W) ** 2
    outs = np.empty((B, N_OUT), dtype=np.float64)
    for c in range(N_CORES):
        t2r = res.results[c]["out_t2"].astype(np.float64)   # [64, 2048]
        t3 = t2r[64:64 + N_OUT].T / SC2
        n3 = t2r[64 + N_OUT] / SC2
        t2 = (
            t2r[0:N_OUT].T / (S_X * S_G)
            + (t2r[16:16 + N_OUT].T + t2r[32:32 + N_OUT].T) / (LO * S_X * S_G)
        )
        mc = m[c * B_CORE:(c + 1) * B_CORE]
        num = mc[:, None] ** 2 * alpha[None, :] + 2 * mc[:, None] * t2 + t3
        den = mc**2 + n3
        outs[c * B_CORE:(c + 1) * B_CORE] = num / den[:, None]

    return np.ascontiguousarray(outs.astype(np.float32))


# revision 15
# speedup vs baseline: 1.1685x; 1.0720x over previous
"""Trainium2 kernel for nn_Circuit_41936060678727.

The reference is a 10-qubit real-amplitude circuit (CNOT ladders + RY
rotations), measured with PauliZ on each wire.  Every gate is linear, so the
circuit collapses to one 784x1024 matrix W with orthonormal rows:

    out[b, p] = sum_z sign_p(z) y_z^2 / ||y||^2,   y = W^T x_b

fp8 trick: x is uniform[0,1), so ~87% of each sample's energy sits in the
all-ones direction u.  Split x = m*u + xt (xt = x - mean, m = u^T x exact on
host).  Then with q = W^T u (||q||=1, exact):

    num_p = m^2 * alpha_p + 2 m * (gamma_p . xt) + sum_z s_pz yt_z^2
    den   = m^2 + ||yt||^2                     (u _|_ xt  =>  q _|_ yt exactly)

where alpha_p = sum_z s_pz q_z^2 (host, exact), gamma_p = W (s_p * q) (host,
quantized hi/lo fp8), yt = W^T xt.  Removing the mean shrinks the operand
~2.8x, which makes e4m3 DoubleRow matmuls (2x PE rate, K=256/pass) accurate
enough: simulated rel err 9.4e-3 vs the 2e-2 gate.

Device per core (batch 2048, groups of 512):
    mm1: yt = Wq^T x_hi          fp8 DR, K=1024(pad), 8 z-chunks, 128 MMs
    sq  = yt^2                   ACT/DVE split, fp8 out (|yt|<=11.6 -> sq<=135)
    mm2: po = [S|1]^T sq         fp8 DR, po strips share one PSUM bank whose
                                 has_written bits are pre-set by a zero matmul
    t2: gamma^T xt hi/lo trio    fp8 DR, M=32/16, transient PSUM tiles
Host: assemble num/den, divide.
"""

import numpy as np
import ml_dtypes

N_QUBITS = 10
DIM = 1 << N_QUBITS          # 1024
N_OUT = 10
D_IN = 784
B_TOTAL = 16384
N_CORES = 8
B_CORE = B_TOTAL // N_CORES  # 2048
GROUP = 512
N_GROUPS = B_CORE // GROUP   # 4
KSUB = 8                     # 1024 contraction rows = 8 subtiles of 128
NZ = 8                       # 1024 output states = 8 chunks of 128

S_X = 2.0                    # x_hi = e4m3(S_X * xt)
S_W = 4.0                    # Wq   = e4m3(S_W * W)   (S_X*S_W=8 keeps sq<240)
S_G = 16.0                   # g_hi = e4m3(S_G * gamma)
LO = 16.0                    # lo-residual upscale

F8 = ml_dtypes.float8_e4m3   # TRN FP8_EXP4 semantics (max 240, inf at 256)


# ----------------------------------------------------------------------------
# Host-side precompute
# ----------------------------------------------------------------------------

def _apply_ry(S, theta, q):
    B = S.shape[0]
    left, right = 1 << q, 1 << (N_QUBITS - q - 1)
    s = S.reshape(B, left, 2, right)
    c, sn = np.cos(theta / 2), np.sin(theta / 2)
    s0 = c * s[:, :, 0] - sn * s[:, :, 1]
    s1 = sn * s[:, :, 0] + c * s[:, :, 1]
    return np.stack([s0, s1], axis=2).reshape(B, DIM)


def _apply_cnot(S, q):
    B = S.shape[0]
    left, right = 1 << q, 1 << (N_QUBITS - q - 2)
    s = S.reshape(B, left, 2, 2, right)
    s = np.concatenate([s[:, :, :1], np.flip(s[:, :, 1:], axis=3)], axis=2)
    return s.reshape(B, DIM)


def _build_W(params):
    """Circuit applied to basis rows e_0..e_783 -> W[784, 1024], fp64."""
    w = np.pi * np.tanh(params.astype(np.float64))
    S = np.zeros((D_IN, DIM), dtype=np.float64)
    S[np.arange(D_IN), np.arange(D_IN)] = 1.0
    for l in range(params.shape[0]):
        for start in (0, 1):
            for i in range(start, N_QUBITS - 1, 2):
                S = _apply_cnot(S, i)
        for i in range(N_QUBITS):
            S = _apply_ry(S, w[l, i], i)
    return S


def _sign_matrix():
    z = np.arange(DIM)
    S = np.zeros((DIM, N_OUT), dtype=np.float64)
    for p in range(N_OUT):
        S[:, p] = 1.0 - 2.0 * ((z >> (N_QUBITS - 1 - p)) & 1)
    return S


def _e4(a):
    return np.asarray(a, np.float32).astype(F8)


def _pack_k_major(a):
    """[1024, cols] -> [128, 8, cols] with [p, s, :] = a[128 s + p, :]."""
    cols = a.shape[1]
    return np.ascontiguousarray(
        a.reshape(KSUB, 128, cols).transpose(1, 0, 2)
    )


# ----------------------------------------------------------------------------
# Bass program (identical SPMD program on all 8 cores)
# ----------------------------------------------------------------------------

_NC_CACHE = {}
TRACE = False
LAST_RESULTS = None


def _build_bass():
    from contextlib import ExitStack

    import concourse.tile as tile
    from concourse import bacc, mybir

    f32 = mybir.dt.float32
    f8 = mybir.dt.float8e4
    bf16 = mybir.dt.bfloat16
    DR = mybir.MatmulPerfMode.DoubleRow

    nc = bacc.Bacc(
        "TRN2", target_bir_lowering=False, debug=False, num_devices=N_CORES
    )
    zsb_d = nc.declare_dram_parameter("zsb", [128, KSUB, 16], bf16, isOutput=False)
    ghl_d = nc.declare_dram_parameter("ghl", [128, KSUB, 32], f8, isOutput=False)
    wrem_d = nc.declare_dram_parameter("wrem", [128, 2, 128], f8, isOutput=False)
    wq_d = nc.declare_dram_parameter("wq", [128, NZ * 6, 128], f8, isOutput=False)
    xh_ds = [
        nc.declare_dram_parameter(f"xh{g}", [128, 7, GROUP], f8, isOutput=False)
        for g in range(N_GROUPS)
    ]
    xl_ds = [
        nc.declare_dram_parameter(f"xl{g}", [128, 7, GROUP], f8, isOutput=False)
        for g in range(N_GROUPS)
    ]
    out_t2_d = nc.declare_dram_parameter("out_t2", [80, B_CORE], f32, isOutput=True)

    def mm(out, lhsT, rhs, start, stop, tile_position=None):
        nc.tensor.matmul(
            out, lhsT=lhsT, rhs=rhs, start=start, stop=stop,
            perf_mode=DR, skip_group_check=True, tile_position=tile_position,
        )

    with ExitStack() as ctx:
        tc = ctx.enter_context(tile.TileContext(nc))
        gz = ctx.enter_context(tc.tile_pool(name="gz", bufs=1))
        xpool = ctx.enter_context(tc.tile_pool(name="x", bufs=1))
        xlpool = ctx.enter_context(tc.tile_pool(name="xl", bufs=1))
        wpool = ctx.enter_context(tc.tile_pool(name="w", bufs=1))
        sqpool = ctx.enter_context(tc.tile_pool(name="sq", bufs=6))
        ybfpool = ctx.enter_context(tc.tile_pool(name="ybf", bufs=3))
        osb = ctx.enter_context(tc.tile_pool(name="osb", bufs=1))
        pypool = ctx.enter_context(tc.tile_pool(name="py", bufs=8, space="PSUM"))

        # small tensors on the gpsimd (SWDGE) ring: lands early, never
        # starves the big sync-ring stream
        ghl = gz.tile([128, KSUB, 32], f8, tag="ghl")
        nc.gpsimd.dma_start(ghl[:], ghl_d[:, :, :])
        zsb = gz.tile([128, KSUB, 16], bf16, tag="zsb")
        nc.gpsimd.dma_start(zsb[:], zsb_d[:, :, :])
        wrem = gz.tile([128, 2, 128], f8, tag="wrem")
        nc.gpsimd.dma_start(wrem[:], wrem_d[:, :, :])
        wz = gz.tile([128, 128], f8, tag="wz")
        nc.vector.memset(wz[:], 0.0)

        # sync ring: W halves interleaved with per-group x, in the order the
        # PE consumes them
        x_sb, xl_sb = [], []
        wq_sb = wpool.tile([128, 48, 128], f8, tag="wq")
        nc.sync.dma_start(wq_sb[:, 0:24, :], wq_d[:, 0:24, :])
        t = xpool.tile([128, 7, GROUP], f8, tag="xh0", name="xh0")
        nc.sync.dma_start(t[:], xh_ds[0][:, :, :])
        x_sb.append(t)
        nc.sync.dma_start(wq_sb[:, 24:48, :], wq_d[:, 24:48, :])
        for g in range(1, N_GROUPS):
            t = xpool.tile([128, 7, GROUP], f8, tag=f"xh{g}", name=f"xh{g}")
            nc.sync.dma_start(t[:], xh_ds[g][:, :, :])
            x_sb.append(t)
        for g in range(N_GROUPS):
            t = xlpool.tile([128, 7, GROUP], f8, tag=f"xl{g}", name=f"xl{g}")
            nc.sync.dma_start(t[:], xl_ds[g][:, :, :])
            xl_sb.append(t)

        def xs(g, p0, s, p1=None):
            t = xp_sb[g // 2]
            c0 = (g % 2) * GROUP
            if p1 is None:
                return t[:, s, c0:c0 + GROUP] if isinstance(s, int) else None
            return t[p0:p1, s, c0:c0 + GROUP]

        def wslice(z, c):
            return wq_sb[:, z * 6 + 2 * c:z * 6 + 2 * c + 2, :]

        out_t2_sb = osb.tile([80, B_CORE], f32, tag="ot2")

        # --- PE warmup on framework const tiles (no DMA dependency) ---
        warm = pypool.tile([128, GROUP], f32, tag="py", name="warm")
        cbf = nc.const_aps.aps[(mybir.dt.bfloat16, 1.0)]
        for _ in range(10):
            nc.tensor.matmul(
                warm[0:1, :], lhsT=cbf, rhs=cbf.to_broadcast([128, GROUP]),
                start=True, stop=True, skip_group_check=True,
            )

        sq_tiles = {}

        def emit_mm1(g):
            # two z-quads: 12 DR passes (K=768) + 4 row-packed K=16 remainders
            for h in range(2):
                pys = [
                    pypool.tile([128, GROUP], f32, tag="py", name=f"py{g}_{4*h+r}")
                    for r in range(4)
                ]
                for c in range(3):
                    for r in range(4):
                        mm(pys[r][:], wslice(4 * h + r, c),
                           x_sb[g][:, 2 * c:2 * c + 2, :],
                           start=(c == 0), stop=False)
                for r in range(4):
                    nc.tensor.matmul(
                        pys[r][:], lhsT=wrem[32 * r:32 * r + 16, h, :],
                        rhs=x_sb[g][32 * r:32 * r + 16, 6, :],
                        start=False, stop=True, skip_group_check=True,
                        tile_position=(32 * r, 0),
                    )
                for r in range(4):
                    z = 4 * h + r
                    pair = z // 2
                    if z % 2 == 0:
                        sq_tiles[(pair, g)] = sqpool.tile(
                            [128, 2, GROUP], bf16, tag="sqb", name=f"sq{pair}_{g}"
                        )
                    dst = sq_tiles[(pair, g)][:, z % 2, :]
                    if z % 2 == 0:
                        nc.scalar.square(dst, pys[r][:])
                    else:
                        ybf = ybfpool.tile([128, GROUP], bf16, tag="ybf",
                                           name=f"ybf{g}_{z}")
                        nc.vector.tensor_copy(ybf[:], pys[r][:])
                        nc.vector.tensor_tensor(dst, ybf[:], ybf[:],
                                                mybir.AluOpType.mult)

        def emit_trio(g):
            # one PSUM bank: zero-matmul sets has_written for the whole bank,
            # then t2a (col grp 0), t2b (col grp 1), mm2 (col grp 2) chains
            # accumulate with start=False; consecutive MMs hit disjoint column
            # groups and run concurrently.
            trio = pypool.tile([128, GROUP], f32, tag="py", name=f"trio{g}")
            gcols = slice(g * GROUP, (g + 1) * GROUP)
            nc.tensor.matmul(
                trio[:], lhsT=wz[:], rhs=x_sb[g][:, 0, :],
                start=True, stop=False, skip_group_check=True,
            )
            for s in range(8):
                if s < 6:
                    nc.tensor.matmul(
                        trio[0:32, :], lhsT=ghl[:, s, 0:32],
                        rhs=x_sb[g][:, s, :],
                        start=False, stop=False, skip_group_check=True,
                        tile_position=(0, 0),
                    )
                    nc.tensor.matmul(
                        trio[32:48, :], lhsT=ghl[:, s, 0:16],
                        rhs=xl_sb[g][:, s, :],
                        start=False, stop=False, skip_group_check=True,
                        tile_position=(0, 32),
                    )
                elif s == 6:  # K=16 tails (rows 768..783)
                    nc.tensor.matmul(
                        trio[0:32, :], lhsT=ghl[0:16, 6, 0:32],
                        rhs=x_sb[g][0:16, 6, :],
                        start=False, stop=True, skip_group_check=True,
                        tile_position=(0, 0),
                    )
                    nc.tensor.matmul(
                        trio[32:48, :], lhsT=ghl[0:16, 6, 0:16],
                        rhs=xl_sb[g][0:16, 6, :],
                        start=False, stop=True, skip_group_check=True,
                        tile_position=(0, 32),
                    )
                sqt = sq_tiles.pop((s // 2, g)) if s % 2 == 0 else sqt_hold
                sqt_hold = sqt
                nc.tensor.matmul(
                    trio[64:80, :], lhsT=zsb[:, s, :], rhs=sqt[:, s % 2, :],
                    start=False, stop=(s == 7), skip_group_check=True,
                    tile_position=(0, 64),
                )
            if g == N_GROUPS - 1:
                nc.scalar.copy(out_t2_sb[64:80, gcols], trio[64:80, :])
                nc.vector.tensor_copy(out_t2_sb[0:48, gcols], trio[0:48, :])
            elif g % 2 == 0:
                nc.scalar.copy(out_t2_sb[0:80, gcols], trio[0:80, :])
            else:
                nc.vector.tensor_copy(out_t2_sb[0:80, gcols], trio[0:80, :])
            nc.gpsimd.dma_start(out_t2_d[:, gcols], out_t2_sb[:, gcols])

        for g in range(N_GROUPS):
            emit_mm1(g)
            emit_trio(g)

    nc.finalize()
    return nc


def _get_nc():
    if "nc" not in _NC_CACHE:
        _NC_CACHE["nc"] = _build_bass()
    return _NC_CACHE["nc"]


# ----------------------------------------------------------------------------
# Entry point
# ----------------------------------------------------------------------------

def kernel(input, params):
    global LAST_RESULTS
    from concourse.bass_utils import run_bass_kernel_spmd

    x = np.asarray(input, dtype=np.float64)
    p = np.asarray(params, dtype=np.float32)
    B = x.shape[0]
    assert B == B_TOTAL and x.shape[1] == D_IN

    W = _build_W(p)                       # [784, 1024] fp64
    S = _sign_matrix()                    # [1024, 10]
    u = np.ones(D_IN) / np.sqrt(D_IN)
    q = W.T @ u                           # [1024]
    alpha = (S * (q**2)[:, None]).sum(axis=0)          # [10]
    gamma = W @ (S * q[:, None])                       # [784, 10]

    # per-sample DC split (host, exact fp64)
    m = x @ u                                           # [B]
    xt = (x - m[:, None] * u[None, :]).astype(np.float32)

    # quantize + pack weights
    Wp = np.zeros((DIM, DIM), dtype=np.float32)
    Wp[:D_IN] = S_W * W.astype(np.float32)
    W8 = _e4(Wp)                                        # [1024, 1024]
    # wq[p, z*6+s, m] = W8[128 s + p, 128 z + m] for s < 6 (K rows 0..767)
    wq_host = np.ascontiguousarray(
        W8.reshape(KSUB, 128, NZ, 128).transpose(1, 2, 0, 3)[:, :, 0:6, :]
        .reshape(128, NZ * 6, 128)
    )
    # wrem[32 r + j, h, m] = W8[768 + j, 128 (4 h + r) + m]  (K rows 768..783)
    wrem_host = np.zeros((128, 2, 128), dtype=W8.dtype)
    for r in range(4):
        for h in range(2):
            wrem_host[32 * r:32 * r + 16, h, :] = W8[768:784,
                                                     128 * (4 * h + r):
                                                     128 * (4 * h + r) + 128]

    G = np.zeros((DIM, 32), dtype=np.float32)
    G[:D_IN, 0:N_OUT] = S_G * gamma
    g_hi = _e4(G)
    g_res = np.zeros((DIM, 32), dtype=np.float32)
    g_res[:, 16:16 + N_OUT] = LO * (
        G[:, 0:N_OUT] - g_hi[:, 0:N_OUT].astype(np.float32)
    )
    ghl_host = _pack_k_major(
        (g_hi.astype(np.float32) + g_res).astype(np.float32)
    )
    ghl_host = _e4(ghl_host)

    Z = np.zeros((DIM, 16), dtype=np.float32)
    Z[:, :N_OUT] = S
    Z[:, N_OUT] = 1.0
    zsb_host = _pack_k_major(Z).astype(ml_dtypes.bfloat16)

    # quantize + pack x (hi/lo): [128, 7, B] with ksub 6 = K rows 768..783
    # at partition strips 0,32,64,96 (replicas feed the row-packed remainder)
    xtT = np.zeros((DIM, B), dtype=np.float32)
    xtT[:D_IN] = S_X * xt.T
    x_hi = _e4(xtT)                                     # [1024, B]
    x_lo = _e4(LO * (xtT - x_hi.astype(np.float32)))

    def pack_x(a):
        out = np.zeros((128, 7, B), dtype=a.dtype)
        out[:, 0:6, :] = a[0:768].reshape(6, 128, B).transpose(1, 0, 2)
        for r in range(4):
            out[32 * r:32 * r + 16, 6, :] = a[768:784]
        return out

    xh_all = pack_x(x_hi)
    xl_all = pack_x(x_lo)

    nc = _get_nc()
    in_maps = []
    for c in range(N_CORES):
        sl = slice(c * B_CORE, (c + 1) * B_CORE)
        m_in = {
            "zsb": zsb_host,
            "ghl": ghl_host,
            "wrem": wrem_host,
            "wq": wq_host,
        }
        for g in range(N_GROUPS):
            gsl = slice(c * B_CORE + g * GROUP, c * B_CORE + (g + 1) * GROUP)
            m_in[f"xh{g}"] = np.ascontiguousarray(xh_all[:, :, gsl])
            m_in[f"xl{g}"] = np.ascontiguousarray(xl_all[:, :, gsl])
        in_maps.append(m_in)

    res = run_bass_kernel_spmd(nc, in_maps, list(range(N_CORES)), trace=TRACE)
    LAST_RESULTS = res

    SC2 = (S_X * S_W) ** 2
    outs = np.empty((B, N_OUT), dtype=np.float64)
    for c in range(N_CORES):
        t2r = res.results[c]["out_t2"].astype(np.float64)   # [64, 2048]
        t3 = t2r[64:64 + N_OUT].T / SC2
        n3 = t2r[64 + N_OUT] / SC2
        t2 = (
            t2r[0:N_OUT].T / (S_X * S_G)
            + (t2r[16:16 + N_OUT].T + t2r[32:32 + N_OUT].T) / (LO * S_X * S_G)
        )
        mc = m[c * B_CORE:(c + 1) * B_CORE]
        num = mc[:, None] ** 2 * alpha[None, :] + 2 * mc[:, None] * t2 + t3
        den = mc**2 + n3
        outs[c * B_CORE:(c + 1) * B_CORE] = num / den[:, None]

    return np.ascontiguousarray(outs.astype(np.float32))


# revision 16
# speedup vs baseline: 1.1948x; 1.0225x over previous
"""Trainium2 kernel for nn_Circuit_41936060678727.

The reference is a 10-qubit real-amplitude circuit (CNOT ladders + RY
rotations), measured with PauliZ on each wire.  Every gate is linear, so the
circuit collapses to one 784x1024 matrix W with orthonormal rows:

    out[b, p] = sum_z sign_p(z) y_z^2 / ||y||^2,   y = W^T x_b

fp8 trick: x is uniform[0,1), so ~87% of each sample's energy sits in the
all-ones direction u.  Split x = m*u + xt (xt = x - mean, m = u^T x exact on
host).  Then with q = W^T u (||q||=1, exact):

    num_p = m^2 * alpha_p + 2 m * (gamma_p . xt) + sum_z s_pz yt_z^2
    den   = m^2 + ||yt||^2                     (u _|_ xt  =>  q _|_ yt exactly)

where alpha_p = sum_z s_pz q_z^2 (host, exact), gamma_p = W (s_p * q) (host,
quantized hi/lo fp8), yt = W^T xt.  Removing the mean shrinks the operand
~2.8x, which makes e4m3 DoubleRow matmuls (2x PE rate, K=256/pass) accurate
enough: simulated rel err 9.4e-3 vs the 2e-2 gate.

Device per core (batch 2048, groups of 512):
    mm1: yt = Wq^T x_hi          fp8 DR, K=1024(pad), 8 z-chunks, 128 MMs
    sq  = yt^2                   ACT/DVE split, fp8 out (|yt|<=11.6 -> sq<=135)
    mm2: po = [S|1]^T sq         fp8 DR, po strips share one PSUM bank whose
                                 has_written bits are pre-set by a zero matmul
    t2: gamma^T xt hi/lo trio    fp8 DR, M=32/16, transient PSUM tiles
Host: assemble num/den, divide.
"""

import numpy as np
import ml_dtypes

N_QUBITS = 10
DIM = 1 << N_QUBITS          # 1024
N_OUT = 10
D_IN = 784
B_TOTAL = 16384
N_CORES = 8
B_CORE = B_TOTAL // N_CORES  # 2048
GROUP = 512
N_GROUPS = B_CORE // GROUP   # 4
KSUB = 8                     # 1024 contraction rows = 8 subtiles of 128
NZ = 8                       # 1024 output states = 8 chunks of 128

S_X = 2.0                    # x_hi = e4m3(S_X * xt)
S_W = 4.0                    # Wq   = e4m3(S_W * W)   (S_X*S_W=8 keeps sq<240)
S_G = 16.0                   # g_hi = e4m3(S_G * gamma)
LO = 16.0                    # lo-residual upscale

F8 = ml_dtypes.float8_e4m3   # TRN FP8_EXP4 semantics (max 240, inf at 256)


# ----------------------------------------------------------------------------
# Host-side precompute
# ----------------------------------------------------------------------------

def _apply_ry(S, theta, q):
    B = S.shape[0]
    left, right = 1 << q, 1 << (N_QUBITS - q - 1)
    s = S.reshape(B, left, 2, right)
    c, sn = np.cos(theta / 2), np.sin(theta / 2)
    s0 = c * s[:, :, 0] - sn * s[:, :, 1]
    s1 = sn * s[:, :, 0] + c * s[:, :, 1]
    return np.stack([s0, s1], axis=2).reshape(B, DIM)


def _apply_cnot(S, q):
    B = S.shape[0]
    left, right = 1 << q, 1 << (N_QUBITS - q - 2)
    s = S.reshape(B, left, 2, 2, right)
    s = np.concatenate([s[:, :, :1], np.flip(s[:, :, 1:], axis=3)], axis=2)
    return s.reshape(B, DIM)


def _build_W(params):
    """Circuit applied to basis rows e_0..e_783 -> W[784, 1024], fp64."""
    w = np.pi * np.tanh(params.astype(np.float64))
    S = np.zeros((D_IN, DIM), dtype=np.float64)
    S[np.arange(D_IN), np.arange(D_IN)] = 1.0
    for l in range(params.shape[0]):
        for start in (0, 1):
            for i in range(start, N_QUBITS - 1, 2):
                S = _apply_cnot(S, i)
        for i in range(N_QUBITS):
            S = _apply_ry(S, w[l, i], i)
    return S


def _sign_matrix():
    z = np.arange(DIM)
    S = np.zeros((DIM, N_OUT), dtype=np.float64)
    for p in range(N_OUT):
        S[:, p] = 1.0 - 2.0 * ((z >> (N_QUBITS - 1 - p)) & 1)
    return S


def _e4(a):
    return np.asarray(a, np.float32).astype(F8)


def _pack_k_major(a):
    """[1024, cols] -> [128, 8, cols] with [p, s, :] = a[128 s + p, :]."""
    cols = a.shape[1]
    return np.ascontiguousarray(
        a.reshape(KSUB, 128, cols).transpose(1, 0, 2)
    )


# ----------------------------------------------------------------------------
# Bass program (identical SPMD program on all 8 cores)
# ----------------------------------------------------------------------------

_NC_CACHE = {}
TRACE = False
LAST_RESULTS = None


def _build_bass():
    from contextlib import ExitStack

    import concourse.tile as tile
    from concourse import bacc, mybir

    f32 = mybir.dt.float32
    f8 = mybir.dt.float8e4
    bf16 = mybir.dt.bfloat16
    DR = mybir.MatmulPerfMode.DoubleRow

    nc = bacc.Bacc(
        "TRN2", target_bir_lowering=False, debug=False, num_devices=N_CORES
    )
    zsb_d = nc.declare_dram_parameter("zsb", [128, KSUB, 16], bf16, isOutput=False)
    ghl_d = nc.declare_dram_parameter("ghl", [128, KSUB, 32], f8, isOutput=False)
    wrem_d = nc.declare_dram_parameter("wrem", [128, 2, 128], f8, isOutput=False)
    wq_d = nc.declare_dram_parameter("wq", [128, NZ * 6, 128], f8, isOutput=False)
    xh_ds = [
        nc.declare_dram_parameter(f"xh{g}", [128, 7, GROUP], f8, isOutput=False)
        for g in range(N_GROUPS)
    ]
    xl_ds = [
        nc.declare_dram_parameter(f"xl{g}", [128, 7, GROUP], f8, isOutput=False)
        for g in range(N_GROUPS)
    ]
    out_t2_d = nc.declare_dram_parameter("out_t2", [80, B_CORE], f32, isOutput=True)

    def mm(out, lhsT, rhs, start, stop, tile_position=None):
        nc.tensor.matmul(
            out, lhsT=lhsT, rhs=rhs, start=start, stop=stop,
            perf_mode=DR, skip_group_check=True, tile_position=tile_position,
        )

    with ExitStack() as ctx:
        tc = ctx.enter_context(tile.TileContext(nc))
        gz = ctx.enter_context(tc.tile_pool(name="gz", bufs=1))
        xpool = ctx.enter_context(tc.tile_pool(name="x", bufs=1))
        xlpool = ctx.enter_context(tc.tile_pool(name="xl", bufs=1))
        wpool = ctx.enter_context(tc.tile_pool(name="w", bufs=1))
        sqpool = ctx.enter_context(tc.tile_pool(name="sq", bufs=6))
        ybfpool = ctx.enter_context(tc.tile_pool(name="ybf", bufs=3))
        osb = ctx.enter_context(tc.tile_pool(name="osb", bufs=1))
        pypool = ctx.enter_context(tc.tile_pool(name="py", bufs=8, space="PSUM"))

        # small tensors on the gpsimd (SWDGE) ring: lands early, never
        # starves the big sync-ring stream
        ghl = gz.tile([128, KSUB, 32], f8, tag="ghl")
        nc.gpsimd.dma_start(ghl[:], ghl_d[:, :, :])
        zsb = gz.tile([128, KSUB, 16], bf16, tag="zsb")
        nc.gpsimd.dma_start(zsb[:], zsb_d[:, :, :])
        wrem = gz.tile([128, 2, 128], f8, tag="wrem")
        nc.gpsimd.dma_start(wrem[:], wrem_d[:, :, :])
        wz = gz.tile([128, 128], f8, tag="wz")
        nc.vector.memset(wz[:], 0.0)

        # sync ring: W halves interleaved with per-group x, in the order the
        # PE consumes them
        x_sb, xl_sb = [], []
        wq_sb = wpool.tile([128, 48, 128], f8, tag="wq")
        nc.sync.dma_start(wq_sb[:, 0:24, :], wq_d[:, 0:24, :])
        t = xpool.tile([128, 7, GROUP], f8, tag="xh0", name="xh0")
        nc.sync.dma_start(t[:], xh_ds[0][:, :, :])
        x_sb.append(t)
        nc.sync.dma_start(wq_sb[:, 24:48, :], wq_d[:, 24:48, :])
        for g in range(1, N_GROUPS):
            t = xpool.tile([128, 7, GROUP], f8, tag=f"xh{g}", name=f"xh{g}")
            nc.sync.dma_start(t[:], xh_ds[g][:, :, :])
            x_sb.append(t)
        for g in range(N_GROUPS):
            t = xlpool.tile([128, 7, GROUP], f8, tag=f"xl{g}", name=f"xl{g}")
            nc.sync.dma_start(t[:], xl_ds[g][:, :, :])
            xl_sb.append(t)

        def xs(g, p0, s, p1=None):
            t = xp_sb[g // 2]
            c0 = (g % 2) * GROUP
            if p1 is None:
                return t[:, s, c0:c0 + GROUP] if isinstance(s, int) else None
            return t[p0:p1, s, c0:c0 + GROUP]

        def wslice(z, c):
            return wq_sb[:, z * 6 + 2 * c:z * 6 + 2 * c + 2, :]

        out_t2_sb = osb.tile([80, B_CORE], f32, tag="ot2")

        # --- PE warmup on framework const tiles (no DMA dependency) ---
        warm = pypool.tile([128, GROUP], f32, tag="py", name="warm")
        cbf = nc.const_aps.aps[(mybir.dt.bfloat16, 1.0)]
        for _ in range(10):
            nc.tensor.matmul(
                warm[0:1, :], lhsT=cbf, rhs=cbf.to_broadcast([128, GROUP]),
                start=True, stop=True, skip_group_check=True,
            )

        sq_tiles = {}

        def emit_mm1(g):
            # two z-quads: 12 DR passes (K=768) + 4 row-packed K=16 remainders
            for h in range(2):
                pys = [
                    pypool.tile([128, GROUP], f32, tag="py", name=f"py{g}_{4*h+r}")
                    for r in range(4)
                ]
                for c in range(3):
                    for r in range(4):
                        mm(pys[r][:], wslice(4 * h + r, c),
                           x_sb[g][:, 2 * c:2 * c + 2, :],
                           start=(c == 0), stop=False)
                for r in range(4):
                    nc.tensor.matmul(
                        pys[r][:], lhsT=wrem[32 * r:32 * r + 16, h, :],
                        rhs=x_sb[g][32 * r:32 * r + 16, 6, :],
                        start=False, stop=True, skip_group_check=True,
                        tile_position=(32 * r, 0),
                    )
                for r in range(4):
                    z = 4 * h + r
                    pair = z // 2
                    if z % 2 == 0:
                        sq_tiles[(pair, g)] = sqpool.tile(
                            [128, 2, GROUP], bf16, tag="sqb", name=f"sq{pair}_{g}"
                        )
                    dst = sq_tiles[(pair, g)][:, z % 2, :]
                    act_z = (z + (1 if g == N_GROUPS - 1 else 0)) % 2 == 0
                    if act_z:
                        nc.scalar.square(dst, pys[r][:])
                    else:
                        ybf = ybfpool.tile([128, GROUP], bf16, tag="ybf",
                                           name=f"ybf{g}_{z}")
                        nc.vector.tensor_copy(ybf[:], pys[r][:])
                        nc.vector.tensor_tensor(dst, ybf[:], ybf[:],
                                                mybir.AluOpType.mult)

        def emit_trio(g):
            # one PSUM bank: zero-matmul sets has_written for the whole bank,
            # then t2a (col grp 0), t2b (col grp 1), mm2 (col grp 2) chains
            # accumulate with start=False; consecutive MMs hit disjoint column
            # groups and run concurrently.
            trio = pypool.tile([128, GROUP], f32, tag="py", name=f"trio{g}")
            gcols = slice(g * GROUP, (g + 1) * GROUP)
            nc.tensor.matmul(
                trio[:], lhsT=wz[:], rhs=x_sb[g][:, 0, :],
                start=True, stop=False, skip_group_check=True,
            )
            for s in range(8):
                if s < 6:
                    nc.tensor.matmul(
                        trio[0:32, :], lhsT=ghl[:, s, 0:32],
                        rhs=x_sb[g][:, s, :],
                        start=False, stop=False, skip_group_check=True,
                        tile_position=(0, 0),
                    )
                    nc.tensor.matmul(
                        trio[32:48, :], lhsT=ghl[:, s, 0:16],
                        rhs=xl_sb[g][:, s, :],
                        start=False, stop=False, skip_group_check=True,
                        tile_position=(0, 32),
                    )
                elif s == 6:  # K=16 tails (rows 768..783)
                    nc.tensor.matmul(
                        trio[0:32, :], lhsT=ghl[0:16, 6, 0:32],
                        rhs=x_sb[g][0:16, 6, :],
                        start=False, stop=True, skip_group_check=True,
                        tile_position=(0, 0),
                    )
                    nc.tensor.matmul(
                        trio[32:48, :], lhsT=ghl[0:16, 6, 0:16],
                        rhs=xl_sb[g][0:16, 6, :],
                        start=False, stop=True, skip_group_check=True,
                        tile_position=(0, 32),
                    )
                sqt = sq_tiles.pop((s // 2, g)) if s % 2 == 0 else sqt_hold
                sqt_hold = sqt
                nc.tensor.matmul(
                    trio[64:80, :], lhsT=zsb[:, s, :], rhs=sqt[:, s % 2, :],
                    start=False, stop=(s == 7), skip_group_check=True,
                    tile_position=(0, 64),
                )
            if g == N_GROUPS - 1:
                nc.scalar.copy(out_t2_sb[64:80, gcols], trio[64:80, :])
                nc.vector.tensor_copy(out_t2_sb[0:48, gcols], trio[0:48, :])
            elif g % 2 == 0:
                nc.scalar.copy(out_t2_sb[0:80, gcols], trio[0:80, :])
            else:
                nc.vector.tensor_copy(out_t2_sb[0:80, gcols], trio[0:80, :])
            nc.sync.dma_start(out_t2_d[:, gcols], out_t2_sb[:, gcols])

        for g in range(N_GROUPS):
            emit_mm1(g)
            emit_trio(g)

    nc.finalize()
    return nc


def _get_nc():
    if "nc" not in _NC_CACHE:
        _NC_CACHE["nc"] = _build_bass()
    return _NC_CACHE["nc"]


# ----------------------------------------------------------------------------
# Entry point
# ----------------------------------------------------------------------------

def kernel(input, params):
    global LAST_RESULTS
    from concourse.bass_utils import run_bass_kernel_spmd

    x = np.asarray(input, dtype=np.float64)
    p = np.asarray(params, dtype=np.float32)
    B = x.shape[0]
    assert B == B_TOTAL and x.shape[1] == D_IN

    W = _build_W(p)                       # [784, 1024] fp64
    S = _sign_matrix()                    # [1024, 10]
    u = np.ones(D_IN) / np.sqrt(D_IN)
    q = W.T @ u                           # [1024]
    alpha = (S * (q**2)[:, None]).sum(axis=0)          # [10]
    gamma = W @ (S * q[:, None])                       # [784, 10]

    # per-sample DC split (host, exact fp64)
    m = x @ u                                           # [B]
    xt = (x - m[:, None] * u[None, :]).astype(np.float32)

    # quantize + pack weights
    Wp = np.zeros((DIM, DIM), dtype=np.float32)
    Wp[:D_IN] = S_W * W.astype(np.float32)
    W8 = _e4(Wp)                                        # [1024, 1024]
    # wq[p, z*6+s, m] = W8[128 s + p, 128 z + m] for s < 6 (K rows 0..767)
    wq_host = np.ascontiguousarray(
        W8.reshape(KSUB, 128, NZ, 128).transpose(1, 2, 0, 3)[:, :, 0:6, :]
        .reshape(128, NZ * 6, 128)
    )
    # wrem[32 r + j, h, m] = W8[768 + j, 128 (4 h + r) + m]  (K rows 768..783)
    wrem_host = np.zeros((128, 2, 128), dtype=W8.dtype)
    for r in range(4):
        for h in range(2):
            wrem_host[32 * r:32 * r + 16, h, :] = W8[768:784,
                                                     128 * (4 * h + r):
                                                     128 * (4 * h + r) + 128]

    G = np.zeros((DIM, 32), dtype=np.float32)
    G[:D_IN, 0:N_OUT] = S_G * gamma
    g_hi = _e4(G)
    g_res = np.zeros((DIM, 32), dtype=np.float32)
    g_res[:, 16:16 + N_OUT] = LO * (
        G[:, 0:N_OUT] - g_hi[:, 0:N_OUT].astype(np.float32)
    )
    ghl_host = _pack_k_major(
        (g_hi.astype(np.float32) + g_res).astype(np.float32)
    )
    ghl_host = _e4(ghl_host)

    Z = np.zeros((DIM, 16), dtype=np.float32)
    Z[:, :N_OUT] = S
    Z[:, N_OUT] = 1.0
    zsb_host = _pack_k_major(Z).astype(ml_dtypes.bfloat16)

    # quantize + pack x (hi/lo): [128, 7, B] with ksub 6 = K rows 768..783
    # at partition strips 0,32,64,96 (replicas feed the row-packed remainder)
    xtT = np.zeros((DIM, B), dtype=np.float32)
    xtT[:D_IN] = S_X * xt.T
    x_hi = _e4(xtT)                                     # [1024, B]
    x_lo = _e4(LO * (xtT - x_hi.astype(np.float32)))

    def pack_x(a):
        out = np.zeros((128, 7, B), dtype=a.dtype)
        out[:, 0:6, :] = a[0:768].reshape(6, 128, B).transpose(1, 0, 2)
        for r in range(4):
            out[32 * r:32 * r + 16, 6, :] = a[768:784]
        return out

    xh_all = pack_x(x_hi)
    xl_all = pack_x(x_lo)

    nc = _get_nc()
    in_maps = []
    for c in range(N_CORES):
        sl = slice(c * B_CORE, (c + 1) * B_CORE)
        m_in = {
            "zsb": zsb_host,
            "ghl": ghl_host,
            "wrem": wrem_host,
            "wq": wq_host,
        }
        for g in range(N_GROUPS):
            gsl = slice(c * B_CORE + g * GROUP, c * B_CORE + (g + 1) * GROUP)
            m_in[f"xh{g}"] = np.ascontiguousarray(xh_all[:, :, gsl])
            m_in[f"xl{g}"] = np.ascontiguousarray(xl_all[:, :, gsl])
        in_maps.append(m_in)

    res = run_bass_kernel_spmd(nc, in_maps, list(range(N_CORES)), trace=TRACE)
    LAST_RESULTS = res

    SC2 = (S_X * S_W) ** 2
    outs = np.empty((B, N_OUT), dtype=np.float64)
    for c in range(N_CORES):
        t2r = res.results[c]["out_t2"].astype(np.float64)   # [64, 2048]
        t3 = t2r[64:64 + N_OUT].T / SC2
        n3 = t2r[64 + N_OUT] / SC2
        t2 = (
            t2r[0:N_OUT].T / (S_X * S_G)
            + (t2r[16:16 + N_OUT].T + t2r[32:32 + N_OUT].T) / (LO * S_X * S_G)
        )
        mc = m[c * B_CORE:(c + 1) * B_CORE]
        num = mc[:, None] ** 2 * alpha[None, :] + 2 * mc[:, None] * t2 + t3
        den = mc**2 + n3
        outs[c * B_CORE:(c + 1) * B_CORE] = num / den[:, None]

    return np.ascontiguousarray(outs.astype(np.float32))
